# revision 1
# baseline (speedup 1.0000x reference)
#!/usr/bin/env python3
"""Trainium2 Bass kernel for nn_DecoderBlock (B=4,T=1024,C=1024,H=16,FFN=C).

Sharding: 8 NeuronCores, zero collectives. Core c owns 512 query tokens of
batch b=c//2 (half h=c%2) and computes the whole decoder block for them
end-to-end: causal self-attention over its batch's 1024 keys (full rectangle
+ additive mask; the host permutes tokens so the owned 512 always sit first,
giving a single SPMD program), cross-attention over the encoder, FFN. The
host splits inputs and reassembles the output.

On-device: natural-layout residual stream [tokens, C] in fp32. LayerNorm
stats+apply run in natural layout (per-partition scalars); the [C, tokens]
operand every projection needs is produced by DMA-engine transposes (bf16),
keeping the PE free. Matmul path is bf16 with fp32 PSUM accumulation and an
fp32 residual stream. Wide-K projection chains are split into two K=64
row-group halves running concurrently on the PE (hides LDWEIGHTS + pipe
drain). Attention runs in S^T layout [tk, tq]: the two heads of a pair are
row-group packed K=64 matmuls; exp runs on the scalar engine straight out
of PSUM (scale=1/8 fused); the causal mask is added into PSUM via an
identity-matmul; softmax sums come from an appended ones-column in V (M=65
matmuls); the division is an approx-reciprocal + gpsimd partition-broadcast
fused multiply. LN gamma/beta are folded into the weights on the host;
projection biases are all zero in this problem (verified at runtime) and
skipped, with a rank-1 matmul fallback otherwise.
"""
import sys
if "/opt/trn_rl_repo" not in sys.path:
    sys.path.insert(0, "/opt/trn_rl_repo")

import numpy as np
import ml_dtypes

import concourse.bass as bass
import concourse.mybir as mybir
import concourse.tile as tile
from concourse import bacc
from concourse import bass_utils

B, T, TE, C, H, HD = 4, 1024, 1024, 1024, 16, 64
NCORES = 8
TOWN = T // 2
EPS = 1e-5
F32 = mybir.dt.float32
BF16 = mybir.dt.bfloat16
AF = mybir.ActivationFunctionType
ALU = mybir.AluOpType
AXX = mybir.AxisListType.X
NT_KV = T // 128
NT_OWN = TOWN // 128
NC_T = C // 128
NHP = H // 2
BF = np.dtype(ml_dtypes.bfloat16)
import os
USE_DMA_T = os.environ.get("USE_DMA_T", "0") == "1"
USE_KSPLIT = os.environ.get("USE_KSPLIT", "0") == "1"


def _ksplit_chain(nc, ps, lhs_tiles, rhs_tiles, n_k, bias_mm=None):
    """Accumulate sum_k lhs[k].T @ rhs[k] into psum `ps` as two concurrent
    K=64 row-group chains. lhs_tiles/rhs_tiles: callables k -> AP."""
    if not USE_KSPLIT:
        for k in range(n_k):
            last = (k == n_k - 1) and bias_mm is None
            nc.tensor.matmul(ps, lhs_tiles(k)[:], rhs_tiles(k)[:],
                             start=(k == 0), stop=last, skip_group_check=True)
        if bias_mm is not None:
            bias_mm()
        return
    for k in range(n_k):
        l = lhs_tiles(k); r = rhs_tiles(k)
        nc.tensor.matmul(ps, l[0:64, :], r[0:64, :], start=(k == 0), stop=False,
                         skip_group_check=True)
    for k in range(n_k):
        l = lhs_tiles(k); r = rhs_tiles(k)
        last = (k == n_k - 1) and bias_mm is None
        nc.tensor.matmul(ps, l[64:128, :], r[64:128, :], start=False, stop=last,
                         skip_group_check=True)
    if bias_mm is not None:
        bias_mm()


def _layer_norm_T(nc, sb, stats, x_tiles, n_t, pfx, eps_ap):
    """LN over n_t natural [128, C] tiles -> NC_T transposed bf16 tiles
    [128, n_t*128] (DMA-engine transposes)."""
    msum = stats.tile([128, n_t], F32, tag="msum", bufs=2, name=f"msum{pfx}")
    m_neg = stats.tile([128, n_t], F32, tag="mneg", bufs=2, name=f"mneg{pfx}")
    var_raw = stats.tile([128, n_t], F32, tag="vraw", bufs=2, name=f"vraw{pfx}")
    lnv = stats.tile([128, n_t], F32, tag="lnv", bufs=2, name=f"lnv{pfx}")
    rstd = stats.tile([128, n_t], F32, tag="rstd", bufs=2, name=f"rstd{pfx}")
    for i in range(n_t):
        nc.vector.reduce_sum(msum[:, i:i + 1], x_tiles[i][:], axis=AXX)
    nc.vector.tensor_scalar_mul(m_neg[:], msum[:], -1.0 / C)
    for i in range(n_t):
        scr = sb.tile([128, C], BF16, tag="sqscr", bufs=2, name=f"scr{pfx}{i}")
        nc.scalar.activation(scr[:], x_tiles[i][:], AF.Square,
                             bias=m_neg[:, i:i + 1], accum_out=var_raw[:, i:i + 1])
    nc.scalar.activation(lnv[:], var_raw[:], AF.Ln, scale=1.0 / C, bias=eps_ap)
    nc.scalar.activation(rstd[:], lnv[:], AF.Exp, scale=-0.5)

    out = [sb.tile([128, n_t * 128], BF16, tag=f"lnT{n_t}", bufs=(10 if n_t == NT_KV else NC_T),
                   name=f"lnT{pfx}{ci}") for ci in range(NC_T)]
    if USE_DMA_T:
        for i in range(n_t):
            nat = sb.tile([128, C], BF16, tag="lnnat", bufs=4, name=f"nat{pfx}{i}")
            nc.vector.tensor_scalar(nat[:], x_tiles[i][:], m_neg[:, i:i + 1],
                                    rstd[:, i:i + 1], op0=ALU.add, op1=ALU.mult)
            for ci in range(NC_T):
                nc.scalar.dma_start_transpose(out[ci][:, i * 128:(i + 1) * 128],
                                              nat[:, ci * 128:(ci + 1) * 128])
        return out
    with tc_ref[0].tile_pool(name=f"lnp{pfx}", bufs=4, space="PSUM") as lnp:
        for g in range(0, n_t, 4):
            gn = min(4, n_t - g)
            ln_nat = []
            for i in range(g, g + gn):
                t = sb.tile([128, C], BF16, tag="lnnat", bufs=6, name=f"nat{pfx}{i}")
                nc.vector.tensor_scalar(t[:], x_tiles[i][:], m_neg[:, i:i + 1],
                                        rstd[:, i:i + 1], op0=ALU.add, op1=ALU.mult)
                ln_nat.append(t)
            for ci in range(NC_T):
                ps = lnp.tile([128, 512], BF16, tag="lnp", name=f"lnps{pfx}{ci}{g}")
                for j in range(gn):
                    nc.tensor.transpose(ps[:, j * 128:(j + 1) * 128],
                                        ln_nat[j][:, ci * 128:(ci + 1) * 128],
                                        ident_ref[0][:])
                nc.vector.tensor_copy(out[ci][:, g * 128:(g + gn) * 128],
                                      ps[:, 0:gn * 128])
    return out


tc_ref = [None]
ident_ref = [None]


def _load_w512(nc, wpool, dram_ap, pfx):
    tiles = []
    for ci in range(NC_T):
        row = []
        for nch in range(2):
            t = wpool.tile([128, 512], BF16, tag="w512", bufs=16,
                           name=f"w{pfx}{ci}_{nch}")
            nc.sync.dma_start(t[:], dram_ap[ci * 128:(ci + 1) * 128,
                                            nch * 512:(nch + 1) * 512])
            row.append(t)
        tiles.append(row)
    return tiles


def _attention(nc, tc, sb, wpool, src_qT, src_kvT, nt_k, w_q, w_k, w_v,
               b_q, b_k, b_v, mask_tiles, ident_t, ones_row, ones16, tag,
               vflag=None, mask_from=0):
    """One MHA. Returns avT: NHP x [128, TOWN] bf16 (softmax-normalized).
    vflag: optional [T,1] f32 sbuf tile; V rows (and their ones column) are
    multiplied by it, implementing all-or-nothing key masking per 128-block.
    mask_tiles[i] is applied (identity-matmul add) only for i >= mask_from."""
    nk_cols = nt_k * 128

    # V projection, natural [tk, d], ones column appended per head (65-stride)
    wv_t = _load_w512(nc, wpool, w_v, f"v{tag}")
    v_sb = []
    for i in range(nt_k):
        vt = sb.tile([128, H * 65], BF16, tag="vsb", bufs=NT_KV,
                     name=f"v{tag}{i}")
        ones_dst = vt[:].rearrange("p (h c) -> p h c", c=65)[:, :, 64:65]
        ones_src = ones16[:].rearrange("p (h c) -> p h c", c=1)
        if vflag is not None:
            nc.vector.tensor_scalar_mul(ones_dst, ones_src,
                                        vflag[:, i:i + 1])
        else:
            nc.vector.tensor_copy(ones_dst, ones_src)
        v_sb.append(vt)
    with tc.tile_pool(name=f"psv{tag}", bufs=4, space="PSUM") as ps_v:
        for nch in range(2):
            for i in range(nt_k):
                ps = ps_v.tile([128, 512], F32, tag="vp", name=f"psv{tag}{nch}{i}")
                bias_mm = None
                if b_v is not None:
                    def bias_mm(ps=ps, nch=nch):
                        nc.tensor.matmul(ps[:], ones_row[:, 0:128],
                                         b_v[:, nch * 512:(nch + 1) * 512],
                                         start=False, stop=True,
                                         skip_group_check=True)
                _ksplit_chain(nc, ps[:],
                              lambda k, i=i: src_kvT[k][:, i * 128:(i + 1) * 128],
                              lambda k, nch=nch: wv_t[k][nch][:],
                              NC_T, bias_mm)
                dst = v_sb[i][:, nch * 520:(nch + 1) * 520].rearrange(
                    "p (h c) -> p h c", c=65)[:, :, 0:64]
                src_ps = ps[:].rearrange("p (h c) -> p h c", c=64)
                if vflag is not None:
                    nc.vector.tensor_scalar_mul(dst, src_ps,
                                                vflag[:, i:i + 1])
                else:
                    nc.vector.tensor_copy(dst, src_ps)

    with (
        tc.tile_pool(name=f"psqk{tag}", bufs=2, space="PSUM") as ps_qk,
        tc.tile_pool(name=f"pssc{tag}", bufs=4, space="PSUM") as ps_sc,
        tc.tile_pool(name=f"psav{tag}", bufs=2, space="PSUM") as ps_av,
    ):
        avT = _attn_heads(nc, sb, wpool, ps_qk, ps_sc, ps_av, src_qT, src_kvT,
                          nt_k, w_q, w_k, b_q, b_k, mask_tiles, ident_t,
                          ones_row, v_sb, tag, mask_from)
    return avT


def _attn_heads(nc, sb, wpool, ps_qk, ps_sc, ps_av, src_qT, src_kvT, nt_k,
                w_q, w_k, b_q, b_k, mask_tiles, ident_t, ones_row, v_sb, tag,
                mask_from=0):
    nk_cols = nt_k * 128
    avT = []
    for hp in range(NHP):
        wq_t, wk_t = [], []
        for ci in range(NC_T):
            tq_ = wpool.tile([128, 128], BF16, tag="w128", bufs=16,
                             name=f"wq{tag}{hp}_{ci}")
            nc.sync.dma_start(tq_[:], w_q[ci * 128:(ci + 1) * 128,
                                          hp * 128:(hp + 1) * 128])
            wq_t.append(tq_)
            tk_ = wpool.tile([128, 128], BF16, tag="w128", bufs=16,
                             name=f"wk{tag}{hp}_{ci}")
            nc.sync.dma_start(tk_[:], w_k[ci * 128:(ci + 1) * 128,
                                          hp * 128:(hp + 1) * 128])
            wk_t.append(tk_)

        qT = sb.tile([128, TOWN], BF16, tag="qT", bufs=3, name=f"qT{tag}{hp}")
        ps = ps_qk.tile([128, TOWN], F32, tag="qk", name=f"psq{tag}{hp}")
        bias_mm = None
        if b_q is not None:
            def bias_mm(ps=ps, hp=hp):
                nc.tensor.matmul(ps[:], b_q[:, hp * 128:(hp + 1) * 128],
                                 ones_row[:], start=False, stop=True,
                                 skip_group_check=True)
        _ksplit_chain(nc, ps[:], lambda k: wq_t[k][:], lambda k: src_qT[k][:],
                      NC_T, bias_mm)
        nc.vector.tensor_copy(qT[:], ps[:])

        kT = sb.tile([128, nk_cols], BF16, tag="kT", bufs=2, name=f"kT{tag}{hp}")
        for half in range(nk_cols // 512):
            ps = ps_qk.tile([128, 512], F32, tag="qk", name=f"psk{tag}{hp}{half}")
            bias_mm = None
            if b_k is not None:
                def bias_mm(ps=ps, hp=hp):
                    nc.tensor.matmul(ps[:], b_k[:, hp * 128:(hp + 1) * 128],
                                     ones_row[:], start=False, stop=True,
                                     skip_group_check=True)
            _ksplit_chain(nc, ps[:], lambda k: wk_t[k][:],
                          lambda k, half=half: src_kvT[k][:, half * 512:(half + 1) * 512],
                          NC_T, bias_mm)
            nc.vector.tensor_copy(kT[:, half * 512:(half + 1) * 512], ps[:])

        at = sb.tile([128, TOWN], BF16, tag="avT", bufs=NHP, name=f"avT{tag}{hp}")
        pav = [ps_av.tile([65, TOWN], F32, tag="av", name=f"psav{tag}{hp}{s}")
               for s in range(2)]
        for i in range(nt_k):
            psc = [ps_sc.tile([128, TOWN], F32, tag="sc", name=f"pssc{tag}{hp}{i}{s}")
                   for s in range(2)]
            has_mask = mask_tiles is not None and i >= mask_from
            for sub in range(2):
                nc.tensor.matmul(psc[sub][:],
                                 kT[sub * 64:(sub + 1) * 64, i * 128:(i + 1) * 128],
                                 qT[sub * 64:(sub + 1) * 64, :],
                                 start=True, stop=not has_mask,
                                 skip_group_check=True)
            if has_mask:
                for sub in range(2):
                    nc.tensor.matmul(psc[sub][:], ident_t[:],
                                     mask_tiles[i - mask_from][:],
                                     start=False, stop=True,
                                     skip_group_check=True)
            for sub in range(2):
                h = hp * 2 + sub
                es = sb.tile([128, TOWN], BF16, tag="exp", bufs=5,
                             name=f"es{tag}{h}{i}")
                nc.scalar.activation(es[:], psc[sub][:], AF.Exp,
                                     scale=float(HD) ** -0.5)
                nc.tensor.matmul(pav[sub][:], v_sb[i][:, h * 65:h * 65 + 65],
                                 es[:], start=(i == 0), stop=(i == nt_k - 1),
                                 skip_group_check=True)
        for sub in range(2):
            h = hp * 2 + sub
            s_h = sb.tile([1, TOWN], F32, tag="sums", bufs=2, name=f"s{tag}{h}")
            nc.vector.tensor_copy(s_h[:], pav[sub][64:65, :])
            rec = sb.tile([1, TOWN], F32, tag="rec", bufs=2, name=f"rec{tag}{h}")
            nc.vector.reciprocal_approx_fast(rec[:], s_h[:])
            r_bc = sb.tile([64, TOWN], F32, tag="rbc", bufs=2, name=f"rb{tag}{h}")
            nc.gpsimd.partition_broadcast(r_bc[:], rec[:])
            nc.vector.tensor_mul(at[sub * 64:(sub + 1) * 64, :],
                                 pav[sub][0:64, :], r_bc[:])
        avT.append(at)
    return avT


def _proj_residual(nc, sb, wpool, ps_pool, lhsT_tiles, w_dram, b_row,
                   resid_tiles, out_tiles, ones_row, tag):
    """out[tq, c'] = lhsT.T @ W + bias + resid (fp32 out)."""
    w_t = _load_w512(nc, wpool, w_dram, f"p{tag}")
    for nch in range(2):
        for tqt in range(NT_OWN):
            ps = ps_pool.tile([128, 512], F32, tag="pr", name=f"pr{tag}{nch}{tqt}")
            bias_mm = None
            if b_row is not None:
                def bias_mm(ps=ps, nch=nch):
                    nc.tensor.matmul(ps[:], ones_row[:, 0:128],
                                     b_row[:, nch * 512:(nch + 1) * 512],
                                     start=False, stop=True,
                                     skip_group_check=True)
            _ksplit_chain(nc, ps[:],
                          lambda k, tqt=tqt: lhsT_tiles[k][:, tqt * 128:(tqt + 1) * 128],
                          lambda k, nch=nch: w_t[k][nch][:],
                          NC_T, bias_mm)
            nc.vector.tensor_add(out_tiles[tqt][:, nch * 512:(nch + 1) * 512],
                                 ps[:], resid_tiles[tqt][:, nch * 512:(nch + 1) * 512])


def build(with_bias=False):
    nc = bacc.Bacc("TRN2", target_bir_lowering=False, debug=False,
                   num_devices=NCORES)
    d_in = {}

    def din(name, shape, dt=BF16):
        d_in[name] = nc.dram_tensor(name, shape, dt, kind="ExternalInput").ap()
        return d_in[name]

    x_kv = din("x_kv", [T, C], F32)
    xe = din("xe", [TE, C])
    maskT = din("maskT", [TOWN, TOWN])
    vflag_d = din("vflag", [128, NT_KV], F32)
    ident = din("ident", [128, 128])
    ones16_d = din("ones16", [128, 16])
    for w in ["wq_sa", "wk_sa", "wv_sa", "wp_sa", "wq_ca", "wk_ca", "wv_ca",
              "wp_ca", "w1", "w2"]:
        din(w, [C, C])
    bias_names = ["bq_sa", "bk_sa", "bv_sa", "bp_sa", "bq_ca", "bk_ca",
                  "bv_ca", "bp_ca", "b1", "b2"]
    if with_bias:
        ones_row_d = din("ones_row", [1, 512])
        for b in bias_names:
            din(b, [1, C])
    out_d = nc.dram_tensor("out", [TOWN, C], F32, kind="ExternalOutput").ap()

    with tile.TileContext(nc) as tc:
        with (
            tc.tile_pool(name="sb", bufs=1) as sb,
            tc.tile_pool(name="stats", bufs=1) as stats,
            tc.tile_pool(name="wpool", bufs=1) as wpool,
        ):
            ident_t = sb.tile([128, 128], BF16, tag="ident", name="identt")
            nc.sync.dma_start(ident_t[:], ident)
            tc_ref[0] = tc
            ident_ref[0] = ident_t
            ones16 = sb.tile([128, 16], BF16, tag="ones16", name="ones16t")
            nc.sync.dma_start(ones16[:], ones16_d)
            eps_ap = sb.tile([128, 1], F32, tag="epsap", name="epst")
            nc.gpsimd.memset(eps_ap[:], EPS)
            if with_bias:
                ones_row = sb.tile([1, 512], BF16, tag="onesrow", name="onesrowt")
                nc.sync.dma_start(ones_row[:], ones_row_d)

                def brow(name):
                    t = sb.tile([1, C], BF16, tag="brow", bufs=4, name=f"br{name}")
                    nc.sync.dma_start(t[:], d_in[name])
                    return t
            else:
                ones_row = None
                brow = lambda name: None

            x_tiles = []
            for i in range(NT_KV):
                t = sb.tile([128, C], F32, tag="xkv", bufs=12, name=f"x{i}")
                nc.sync.dma_start(t[:], x_kv[i * 128:(i + 1) * 128, :])
                x_tiles.append(t)
            mask_tiles = []
            for i in range(NT_OWN):
                t = sb.tile([128, TOWN], BF16, tag="mask", bufs=NT_OWN,
                            name=f"mask{i}")
                nc.sync.dma_start(t[:], maskT[i * 128:(i + 1) * 128, :])
                mask_tiles.append(t)
            vflag = sb.tile([128, NT_KV], F32, tag="vflag", name="vflagt")
            nc.sync.dma_start(vflag[:], vflag_d)

            # ---------------- SA (+ ln2 emitted early for overlap) --------
            ln1T = _layer_norm_T(nc, sb, stats, x_tiles, NT_KV, "l1", eps_ap[:])
            ln1T_own = [t[:, TOWN:] for t in ln1T]  # own tokens = rows 512..1023
            xe_tiles = []
            for i in range(NT_KV):
                t = sb.tile([128, C], BF16, tag="xkv", bufs=12, name=f"xe{i}")
                nc.sync.dma_start(t[:], xe[i * 128:(i + 1) * 128, :])
                xe_tiles.append(t)
            avT = _attention(nc, tc, sb, wpool, ln1T_own, ln1T, NT_KV,
                             d_in["wq_sa"], d_in["wk_sa"], d_in["wv_sa"],
                             brow("bq_sa"), brow("bk_sa"), brow("bv_sa"),
                             mask_tiles, ident_t, ones_row, ones16, "sa",
                             vflag=vflag, mask_from=NT_OWN)
            ln2T = _layer_norm_T(nc, sb, stats, xe_tiles, NT_KV, "l2", eps_ap[:])
            x1 = [sb.tile([128, C], F32, tag="res1", bufs=NT_OWN, name=f"x1_{i}")
                  for i in range(NT_OWN)]
            with tc.tile_pool(name="pspr1", bufs=4, space="PSUM") as ps_pr:
                _proj_residual(nc, sb, wpool, ps_pr, avT, d_in["wp_sa"],
                               brow("bp_sa"), x_tiles[NT_OWN:], x1,
                               ones_row, "sa")

            # ---------------- CA ----------------
            ln3T = _layer_norm_T(nc, sb, stats, x1, NT_OWN, "l3", eps_ap[:])
            avT2 = _attention(nc, tc, sb, wpool, ln3T, ln2T, NT_KV,
                              d_in["wq_ca"], d_in["wk_ca"], d_in["wv_ca"],
                              brow("bq_ca"), brow("bk_ca"), brow("bv_ca"),
                              None, ident_t, ones_row, ones16, "ca")
            x2 = [sb.tile([128, C], F32, tag="res2", bufs=NT_OWN, name=f"x2_{i}")
                  for i in range(NT_OWN)]
            with tc.tile_pool(name="pspr2", bufs=4, space="PSUM") as ps_pr:
                _proj_residual(nc, sb, wpool, ps_pr, avT2, d_in["wp_ca"],
                               brow("bp_ca"), x1, x2, ones_row, "ca")

            # ---------------- FFN ----------------
            ln4T = _layer_norm_T(nc, sb, stats, x2, NT_OWN, "l4", eps_ap[:])
            b1r = brow("b1")
            w1_t = _load_w512(nc, wpool, d_in["w1"], "w1")
            gT = []
            with tc.tile_pool(name="psh", bufs=4, space="PSUM") as ps_h:
                for hidt in range(NC_T):
                    ps = ps_h.tile([128, TOWN], F32, tag="h", name=f"psh{hidt}")
                    bias_mm = None
                    if b1r is not None:
                        def bias_mm(ps=ps, hidt=hidt):
                            nc.tensor.matmul(
                                ps[:], b1r[:, hidt * 128:(hidt + 1) * 128],
                                ones_row[:], start=False, stop=True,
                                skip_group_check=True)
                    _ksplit_chain(
                        nc, ps[:],
                        lambda k, hidt=hidt: w1_t[k][hidt // 4][
                            :, (hidt % 4) * 128:(hidt % 4 + 1) * 128],
                        lambda k: ln4T[k][:], NC_T, bias_mm)
                    g = sb.tile([128, TOWN], BF16, tag="gT", bufs=NC_T,
                                name=f"g{hidt}")
                    nc.scalar.activation(g[:], ps[:], AF.Gelu)
                    gT.append(g)
            out_sb = [sb.tile([128, C], F32, tag="res1", bufs=NT_OWN,
                              name=f"osb{i}") for i in range(NT_OWN)]
            with tc.tile_pool(name="psf", bufs=4, space="PSUM") as ps_f:
                _proj_residual(nc, sb, wpool, ps_f, gT, d_in["w2"],
                               brow("b2"), x2, out_sb, ones_row, "f")
            for tqt in range(NT_OWN):
                nc.sync.dma_start(out_d[tqt * 128:(tqt + 1) * 128, :],
                                  out_sb[tqt][:])
    nc.compile()
    return nc


_CACHED = {}


def _get_nc(with_bias):
    if with_bias not in _CACHED:
        _CACHED[with_bias] = build(with_bias)
    return _CACHED[with_bias]


def _stack_heads(w):
    return np.ascontiguousarray(np.transpose(np.asarray(w), (1, 0, 2))
                                .reshape(C, H * HD))


def prepare_in_maps(inputs):
    inp = {k: np.asarray(v, dtype=np.float32) for k, v in inputs.items()}
    g1, be1 = inp["g1"], inp["be1"]
    g2, be2 = inp["g2"], inp["be2"]
    g3, be3 = inp["g3"], inp["be3"]
    g4, be4 = inp["g4"], inp["be4"]

    wq_sa = _stack_heads(inp["Wq_sa"]); wk_sa = _stack_heads(inp["Wk_sa"])
    wv_sa = _stack_heads(inp["Wv_sa"])
    wq_ca = _stack_heads(inp["Wq_ca"]); wk_ca = _stack_heads(inp["Wk_ca"])
    wv_ca = _stack_heads(inp["Wv_ca"])

    biases = {
        "bq_sa": be1 @ wq_sa, "bk_sa": be1 @ wk_sa, "bv_sa": be1 @ wv_sa,
        "bp_sa": inp["bp_sa"],
        "bq_ca": be3 @ wq_ca, "bk_ca": be2 @ wk_ca, "bv_ca": be2 @ wv_ca,
        "bp_ca": inp["bp_ca"],
        "b1": inp["b1"] + be4 @ inp["W1"], "b2": inp["b2"],
    }
    with_bias = any(np.abs(v).max() > 0 for v in biases.values())

    shared = {
        "ident": np.eye(128, dtype=np.float32),
        "ones16": np.ones((128, 16), np.float32),
        "wq_sa": g1[:, None] * wq_sa,
        "wk_sa": g1[:, None] * wk_sa,
        "wv_sa": g1[:, None] * wv_sa,
        "wp_sa": inp["Wp_sa"],
        "wq_ca": g3[:, None] * wq_ca,
        "wk_ca": g2[:, None] * wk_ca,
        "wv_ca": g2[:, None] * wv_ca,
        "wp_ca": inp["Wp_ca"],
        "w1": g4[:, None] * inp["W1"],
        "w2": inp["W2"],
    }
    if with_bias:
        shared["ones_row"] = np.ones((1, 512), np.float32)
        for k, v in biases.items():
            shared[k] = v.reshape(1, C)
    shared = {k: np.ascontiguousarray(v.astype(BF)) for k, v in shared.items()}

    x = inp["x"]; xe = inp["x_encode"]
    in_maps = []
    for core in range(NCORES):
        b = core // 2
        half = core % 2
        own = slice(half * TOWN, (half + 1) * TOWN)
        other = slice((1 - half) * TOWN, (2 - half) * TOWN)
        # kv layout: [other half | own half]; own queries are rows 512..1023
        x_perm = np.concatenate([x[b, other, :], x[b, own, :]], axis=0)
        # other half: fully visible for half=1 (keys before queries), fully
        # hidden for half=0 -> v-flag 0/1; own half: shared triangle mask
        vf = np.zeros(T, np.float32)
        vf[:TOWN] = 1.0 if half == 1 else 0.0
        vf[TOWN:] = 1.0
        vf = np.ascontiguousarray(vf.reshape(NT_KV, 128).T)
        tl = np.arange(TOWN)
        m = np.where(tl[:, None] <= tl[None, :], 0.0, -30000.0)
        im = dict(shared)
        im["x_kv"] = np.ascontiguousarray(x_perm, dtype=np.float32)
        im["xe"] = np.ascontiguousarray(xe[b].astype(BF))
        im["maskT"] = np.ascontiguousarray(m.astype(BF))
        im["vflag"] = vf
        in_maps.append(im)
    return in_maps, with_bias


def run(inputs, trace=False, **kw):
    in_maps, with_bias = prepare_in_maps(inputs)
    nc = _get_nc(with_bias)
    r = bass_utils.run_bass_kernel_spmd(nc, in_maps, core_ids=list(range(NCORES)),
                                        trace=trace, **kw)
    out = np.empty((B, T, C), np.float32)
    for core in range(NCORES):
        b = core // 2
        half = core % 2
        out[b, half * TOWN:(half + 1) * TOWN, :] = r.results[core]["out"]
    return out, r


def kernel(**inputs):
    out, _ = run(inputs)
    return out


if __name__ == "__main__":
    build()
    print("build ok")



# revision 15
# speedup vs baseline: 1.4830x; 1.4830x over previous
#!/usr/bin/env python3
"""Trainium2 Bass kernel for nn_DecoderBlock (B=4,T=1024,C=1024,H=16,FFN=C).

Sharding: 8 NeuronCores, zero collectives. Core c owns 512 query tokens of
batch b=c//2 (half h=c%2) and computes the whole decoder block for them
end-to-end: causal self-attention over its batch's 1024 keys (full rectangle
+ additive mask; the host permutes tokens so the owned 512 always sit first,
giving a single SPMD program), cross-attention over the encoder, FFN. The
host splits inputs and reassembles the output.

On-device: natural-layout residual stream [tokens, C] in fp32. LayerNorm
stats+apply run in natural layout (per-partition scalars); the [C, tokens]
operand every projection needs is produced by DMA-engine transposes (bf16),
keeping the PE free. Matmul path is bf16 with fp32 PSUM accumulation and an
fp32 residual stream. Wide-K projection chains are split into two K=64
row-group halves running concurrently on the PE (hides LDWEIGHTS + pipe
drain). Attention runs in S^T layout [tk, tq]: the two heads of a pair are
row-group packed K=64 matmuls; exp runs on the scalar engine straight out
of PSUM (scale=1/8 fused); the causal mask is added into PSUM via an
identity-matmul; softmax sums come from an appended ones-column in V (M=65
matmuls); the division is an approx-reciprocal + gpsimd partition-broadcast
fused multiply. LN gamma/beta are folded into the weights on the host;
projection biases are all zero in this problem (verified at runtime) and
skipped, with a rank-1 matmul fallback otherwise.
"""
import sys
if "/opt/trn_rl_repo" not in sys.path:
    sys.path.insert(0, "/opt/trn_rl_repo")

import numpy as np
import ml_dtypes

import concourse.bass as bass
import concourse.mybir as mybir
import concourse.tile as tile
from concourse import bacc
from concourse import bass_utils

B, T, TE, C, H, HD = 4, 1024, 1024, 1024, 16, 64
NCORES = 8
TOWN = T // 2
EPS = 1e-5
F32 = mybir.dt.float32
BF16 = mybir.dt.bfloat16
AF = mybir.ActivationFunctionType
ALU = mybir.AluOpType
AXX = mybir.AxisListType.X
NT_KV = T // 128
NT_OWN = TOWN // 128
NC_T = C // 128
NHP = H // 2
BF = np.dtype(ml_dtypes.bfloat16)
import os
USE_DMA_T = os.environ.get("USE_DMA_T", "0") == "1"
USE_KSPLIT = os.environ.get("USE_KSPLIT", "0") == "1"


def _ksplit_chain(nc, ps, lhs_tiles, rhs_tiles, n_k, bias_mm=None):
    """Accumulate sum_k lhs[k].T @ rhs[k] into psum `ps` as two concurrent
    K=64 row-group chains. lhs_tiles/rhs_tiles: callables k -> AP."""
    if not USE_KSPLIT:
        for k in range(n_k):
            last = (k == n_k - 1) and bias_mm is None
            nc.tensor.matmul(ps, lhs_tiles(k)[:], rhs_tiles(k)[:],
                             start=(k == 0), stop=last, skip_group_check=True)
        if bias_mm is not None:
            bias_mm()
        return
    for k in range(n_k):
        l = lhs_tiles(k); r = rhs_tiles(k)
        nc.tensor.matmul(ps, l[0:64, :], r[0:64, :], start=(k == 0), stop=False,
                         skip_group_check=True)
    for k in range(n_k):
        l = lhs_tiles(k); r = rhs_tiles(k)
        last = (k == n_k - 1) and bias_mm is None
        nc.tensor.matmul(ps, l[64:128, :], r[64:128, :], start=False, stop=last,
                         skip_group_check=True)
    if bias_mm is not None:
        bias_mm()


def _layer_norm_T(nc, sb, stats, x_tiles, n_t, pfx, eps_ap):
    """LN over n_t natural [128, C] tiles -> NC_T transposed bf16 tiles
    [128, n_t*128]. Stats chains are per-tile so tile i's transposes are
    unblocked as soon as tile i's data (not the whole LN) is ready."""
    out = [sb.tile([128, n_t * 128], BF16, tag=f"lnT{n_t}", bufs=(10 if n_t == NT_KV else NC_T),
                   name=f"lnT{pfx}{ci}") for ci in range(NC_T)]
    with tc_ref[0].tile_pool(name=f"lnp{pfx}", bufs=4, space="PSUM") as lnp:
        for g in range(0, n_t, 2):
            gn = min(2, n_t - g)
            ln_nat = []
            for i in range(g, g + gn):
                msum = stats.tile([128, 1], F32, tag="msum", bufs=8,
                                  name=f"msum{pfx}{i}")
                m_neg = stats.tile([128, 1], F32, tag="mneg", bufs=8,
                                   name=f"mneg{pfx}{i}")
                var_raw = stats.tile([128, 1], F32, tag="vraw", bufs=8,
                                     name=f"vraw{pfx}{i}")
                rstd = stats.tile([128, 1], F32, tag="rstd", bufs=8,
                                  name=f"rstd{pfx}{i}")
                nc.vector.reduce_sum(msum[:], x_tiles[i][:], axis=AXX)
                nc.vector.tensor_scalar_mul(m_neg[:], msum[:], -1.0 / C)
                scr = sb.tile([128, C], BF16, tag="sqscr", bufs=2,
                              name=f"scr{pfx}{i}")
                lnv = stats.tile([128, 1], F32, tag="lnv", bufs=8,
                                 name=f"lnv{pfx}{i}")
                nc.scalar.activation(scr[:], x_tiles[i][:], AF.Square,
                                     bias=m_neg[:], accum_out=var_raw[:])
                nc.scalar.activation(lnv[:], var_raw[:], AF.Ln,
                                     scale=1.0 / C, bias=eps_ap)
                nc.scalar.activation(rstd[:], lnv[:], AF.Exp, scale=-0.5)
                t = sb.tile([128, C], BF16, tag="lnnat", bufs=6, name=f"nat{pfx}{i}")
                nc.vector.tensor_scalar(t[:], x_tiles[i][:], m_neg[:],
                                        rstd[:], op0=ALU.add, op1=ALU.mult)
                ln_nat.append(t)
            for ci in range(NC_T):
                ps = lnp.tile([128, 256], BF16, tag="lnp", name=f"lnps{pfx}{ci}{g}")
                for j in range(gn):
                    nc.tensor.transpose(ps[:, j * 128:(j + 1) * 128],
                                        ln_nat[j][:, ci * 128:(ci + 1) * 128],
                                        ident_ref[0][:])
                nc.vector.tensor_copy(out[ci][:, g * 128:(g + gn) * 128],
                                      ps[:, 0:gn * 128])
    return out


tc_ref = [None]
ident_ref = [None]


def _load_w512(nc, wpool, dram_ap, pfx):
    """dram_ap is host-packed [NC_T*2*128, 512]: block ci*2+nch holds the
    [128, 512] tile (contiguous 128 KiB -> large DMA descriptors)."""
    tiles = []
    for ci in range(NC_T):
        row = []
        for nch in range(2):
            b = ci * 2 + nch
            t = wpool.tile([128, 512], BF16, tag="w512", bufs=16,
                           name=f"w{pfx}{ci}_{nch}")
            nc.sync.dma_start(t[:], dram_ap[b * 128:(b + 1) * 128, :])
            row.append(t)
        tiles.append(row)
    return tiles


def _attention(nc, tc, sb, wpool, src_qT, src_kvT, nt_k, w_qk, w_v,
               b_q, b_k, b_v, mask_tiles, ident_t, ones_row, ones16, tag,
               vflag=None, mask_from=0):
    """One MHA. Returns avT: NHP x [128, TOWN] bf16 (softmax-normalized).
    vflag: optional [T,1] f32 sbuf tile; V rows (and their ones column) are
    multiplied by it, implementing all-or-nothing key masking per 128-block.
    mask_tiles[i] is applied (identity-matmul add) only for i >= mask_from."""
    nk_cols = nt_k * 128

    # V projection, natural [tk, d], ones column appended per head (65-stride)
    wv_t = _load_w512(nc, wpool, w_v, f"v{tag}")
    v_sb = []
    for i in range(nt_k):
        vt = sb.tile([128, H * 65], BF16, tag="vsb", bufs=NT_KV,
                     name=f"v{tag}{i}")
        ones_dst = vt[:].rearrange("p (h c) -> p h c", c=65)[:, :, 64:65]
        ones_src = ones16[:].rearrange("p (h c) -> p h c", c=1)
        if vflag is not None:
            nc.vector.tensor_scalar_mul(ones_dst, ones_src,
                                        vflag[:, i:i + 1])
        else:
            nc.vector.tensor_copy(ones_dst, ones_src)
        v_sb.append(vt)
    with tc.tile_pool(name=f"psv{tag}", bufs=4, space="PSUM") as ps_v:
        for nch in range(2):
            for i in range(nt_k):
                ps = ps_v.tile([128, 512], F32, tag="vp", name=f"psv{tag}{nch}{i}")
                bias_mm = None
                if b_v is not None:
                    def bias_mm(ps=ps, nch=nch):
                        nc.tensor.matmul(ps[:], ones_row[:, 0:128],
                                         b_v[:, nch * 512:(nch + 1) * 512],
                                         start=False, stop=True,
                                         skip_group_check=True)
                _ksplit_chain(nc, ps[:],
                              lambda k, i=i: src_kvT[k][:, i * 128:(i + 1) * 128],
                              lambda k, nch=nch: wv_t[k][nch][:],
                              NC_T, bias_mm)
                dst = v_sb[i][:, nch * 520:(nch + 1) * 520].rearrange(
                    "p (h c) -> p h c", c=65)[:, :, 0:64]
                src_ps = ps[:].rearrange("p (h c) -> p h c", c=64)
                if vflag is not None:
                    nc.vector.tensor_scalar_mul(dst, src_ps,
                                                vflag[:, i:i + 1])
                else:
                    nc.vector.tensor_copy(dst, src_ps)

    with (
        tc.tile_pool(name=f"psqk{tag}", bufs=2, space="PSUM") as ps_qk,
        tc.tile_pool(name=f"pssc{tag}", bufs=4, space="PSUM") as ps_sc,
        tc.tile_pool(name=f"psav{tag}", bufs=2, space="PSUM") as ps_av,
    ):
        avT = _attn_heads(nc, sb, wpool, ps_qk, ps_sc, ps_av, src_qT, src_kvT,
                          nt_k, w_qk, b_q, b_k, mask_tiles, ident_t,
                          ones_row, v_sb, tag, mask_from)
    return avT


def _attn_heads(nc, sb, wpool, ps_qk, ps_sc, ps_av, src_qT, src_kvT, nt_k,
                w_qk, b_q, b_k, mask_tiles, ident_t, ones_row, v_sb, tag,
                mask_from=0):
    nk_cols = nt_k * 128
    avT = []
    for hp in range(NHP):
        # host-packed [8192, 256]: block hp*8+ci = [wq | wk] for (hp, ci)
        wq_t, wk_t = [], []
        for ci in range(NC_T):
            b = hp * NC_T + ci
            tqk = wpool.tile([128, 256], BF16, tag="w256", bufs=16,
                             name=f"wqk{tag}{hp}_{ci}")
            nc.sync.dma_start(tqk[:], w_qk[b * 128:(b + 1) * 128, :])
            wq_t.append(tqk[:, 0:128])
            wk_t.append(tqk[:, 128:256])

        qT = sb.tile([128, TOWN], BF16, tag="qT", bufs=3, name=f"qT{tag}{hp}")
        ps = ps_qk.tile([128, TOWN], F32, tag="qk", name=f"psq{tag}{hp}")
        bias_mm = None
        if b_q is not None:
            def bias_mm(ps=ps, hp=hp):
                nc.tensor.matmul(ps[:], b_q[:, hp * 128:(hp + 1) * 128],
                                 ones_row[:], start=False, stop=True,
                                 skip_group_check=True)
        _ksplit_chain(nc, ps[:], lambda k: wq_t[k][:], lambda k: src_qT[k][:],
                      NC_T, bias_mm)
        nc.vector.tensor_copy(qT[:], ps[:])

        kT = sb.tile([128, nk_cols], BF16, tag="kT", bufs=2, name=f"kT{tag}{hp}")
        for half in range(nk_cols // 512):
            ps = ps_qk.tile([128, 512], F32, tag="qk", name=f"psk{tag}{hp}{half}")
            bias_mm = None
            if b_k is not None:
                def bias_mm(ps=ps, hp=hp):
                    nc.tensor.matmul(ps[:], b_k[:, hp * 128:(hp + 1) * 128],
                                     ones_row[:], start=False, stop=True,
                                     skip_group_check=True)
            _ksplit_chain(nc, ps[:], lambda k: wk_t[k][:],
                          lambda k, half=half: src_kvT[k][:, half * 512:(half + 1) * 512],
                          NC_T, bias_mm)
            nc.vector.tensor_copy(kT[:, half * 512:(half + 1) * 512], ps[:])

        at = sb.tile([128, TOWN], BF16, tag="avT", bufs=NHP, name=f"avT{tag}{hp}")
        pav = [ps_av.tile([65, TOWN], F32, tag="av", name=f"psav{tag}{hp}{s}")
               for s in range(2)]
        for i in range(nt_k):
            psc = [ps_sc.tile([128, TOWN], F32, tag="sc", name=f"pssc{tag}{hp}{i}{s}")
                   for s in range(2)]
            has_mask = mask_tiles is not None and i >= mask_from
            # Causal own-key tile i': query cols < i'*128 are fully hidden
            # (skip), col block i' is the triangular diagonal (add tri mask),
            # cols > (i'+1)*128 fully visible.
            j0 = (i - mask_from) * 128 if has_mask else 0
            for sub in range(2):
                nc.tensor.matmul(psc[sub][:, j0:TOWN],
                                 kT[sub * 64:(sub + 1) * 64, i * 128:(i + 1) * 128],
                                 qT[sub * 64:(sub + 1) * 64, j0:TOWN],
                                 start=True, stop=not has_mask,
                                 skip_group_check=True)
            if has_mask:
                for sub in range(2):
                    nc.tensor.matmul(psc[sub][:, j0:j0 + 128], ident_t[:],
                                     mask_tiles[0][:],
                                     start=False, stop=True,
                                     skip_group_check=True)
            for sub in range(2):
                h = hp * 2 + sub
                es = sb.tile([128, TOWN], BF16, tag="exp", bufs=5,
                             name=f"es{tag}{h}{i}")
                nc.scalar.activation(es[:, j0:TOWN], psc[sub][:, j0:TOWN],
                                     AF.Exp, scale=float(HD) ** -0.5)
                nc.tensor.matmul(pav[sub][:, j0:TOWN],
                                 v_sb[i][:, h * 65:h * 65 + 65],
                                 es[:, j0:TOWN], start=(i == 0),
                                 stop=(i == nt_k - 1),
                                 skip_group_check=True)
        for sub in range(2):
            h = hp * 2 + sub
            s_h = sb.tile([1, TOWN], F32, tag="sums", bufs=2, name=f"s{tag}{h}")
            nc.vector.tensor_copy(s_h[:], pav[sub][64:65, :])
            rec = sb.tile([1, TOWN], F32, tag="rec", bufs=2, name=f"rec{tag}{h}")
            nc.vector.reciprocal_approx_fast(rec[:], s_h[:])
            r_bc = sb.tile([64, TOWN], F32, tag="rbc", bufs=2, name=f"rb{tag}{h}")
            nc.gpsimd.partition_broadcast(r_bc[:], rec[:])
            nc.vector.tensor_mul(at[sub * 64:(sub + 1) * 64, :],
                                 pav[sub][0:64, :], r_bc[:])
        avT.append(at)
    return avT


def _proj_residual(nc, sb, wpool, ps_pool, lhsT_tiles, w_dram, b_row,
                   resid_tiles, out_tiles, ones_row, tag):
    """out[tq, c'] = lhsT.T @ W + bias + resid (fp32 out)."""
    w_t = _load_w512(nc, wpool, w_dram, f"p{tag}")
    for nch in range(2):
        for tqt in range(NT_OWN):
            ps = ps_pool.tile([128, 512], F32, tag="pr", name=f"pr{tag}{nch}{tqt}")
            bias_mm = None
            if b_row is not None:
                def bias_mm(ps=ps, nch=nch):
                    nc.tensor.matmul(ps[:], ones_row[:, 0:128],
                                     b_row[:, nch * 512:(nch + 1) * 512],
                                     start=False, stop=True,
                                     skip_group_check=True)
            _ksplit_chain(nc, ps[:],
                          lambda k, tqt=tqt: lhsT_tiles[k][:, tqt * 128:(tqt + 1) * 128],
                          lambda k, nch=nch: w_t[k][nch][:],
                          NC_T, bias_mm)
            nc.vector.tensor_add(out_tiles[tqt][:, nch * 512:(nch + 1) * 512],
                                 ps[:], resid_tiles[tqt][:, nch * 512:(nch + 1) * 512])


def build(with_bias=False):
    nc = bacc.Bacc("TRN2", target_bir_lowering=False, debug=False,
                   num_devices=NCORES)
    d_in = {}

    def din(name, shape, dt=BF16):
        d_in[name] = nc.dram_tensor(name, shape, dt, kind="ExternalInput").ap()
        return d_in[name]

    x_kv = din("x_kv", [T, C], F32)
    xe = din("xe", [TE, C])
    maskT = din("maskT", [128, 128])
    vflag_d = din("vflag", [128, NT_KV], F32)
    ident = din("ident", [128, 128])
    ones16_d = din("ones16", [128, 16])
    for w in ["wv_sa", "wp_sa", "wv_ca", "wp_ca", "w1", "w2"]:
        din(w, [NC_T * 2 * 128, 512])
    for w in ["wqk_sa", "wqk_ca"]:
        din(w, [NHP * NC_T * 128, 256])
    bias_names = ["bq_sa", "bk_sa", "bv_sa", "bp_sa", "bq_ca", "bk_ca",
                  "bv_ca", "bp_ca", "b1", "b2"]
    if with_bias:
        ones_row_d = din("ones_row", [1, 512])
        for b in bias_names:
            din(b, [1, C])
    out_d = nc.dram_tensor("out", [TOWN, C], F32, kind="ExternalOutput").ap()

    with tile.TileContext(nc) as tc:
        with (
            tc.tile_pool(name="sb", bufs=1) as sb,
            tc.tile_pool(name="stats", bufs=1) as stats,
            tc.tile_pool(name="wpool", bufs=1) as wpool,
        ):
            ident_t = sb.tile([128, 128], BF16, tag="ident", name="identt")
            nc.sync.dma_start(ident_t[:], ident)
            tc_ref[0] = tc
            ident_ref[0] = ident_t
            ones16 = sb.tile([128, 16], BF16, tag="ones16", name="ones16t")
            nc.sync.dma_start(ones16[:], ones16_d)
            eps_ap = sb.tile([128, 1], F32, tag="epsap", name="epst")
            nc.gpsimd.memset(eps_ap[:], EPS)
            if with_bias:
                ones_row = sb.tile([1, 512], BF16, tag="onesrow", name="onesrowt")
                nc.sync.dma_start(ones_row[:], ones_row_d)

                def brow(name):
                    t = sb.tile([1, C], BF16, tag="brow", bufs=4, name=f"br{name}")
                    nc.sync.dma_start(t[:], d_in[name])
                    return t
            else:
                ones_row = None
                brow = lambda name: None

            x_tiles = []
            for i in range(NT_KV):
                t = sb.tile([128, C], F32, tag="xkv", bufs=12, name=f"x{i}")
                nc.sync.dma_start(t[:], x_kv[i * 128:(i + 1) * 128, :])
                x_tiles.append(t)
            tri = sb.tile([128, 128], BF16, tag="mask", name="tri")
            nc.sync.dma_start(tri[:], maskT)
            mask_tiles = [tri]
            vflag = sb.tile([128, NT_KV], F32, tag="vflag", name="vflagt")
            nc.sync.dma_start(vflag[:], vflag_d)

            # ---------------- SA (+ ln2 emitted early for overlap) --------
            ln1T = _layer_norm_T(nc, sb, stats, x_tiles, NT_KV, "l1", eps_ap[:])
            ln1T_own = [t[:, TOWN:] for t in ln1T]  # own tokens = rows 512..1023
            xe_tiles = []
            for i in range(NT_KV):
                t = sb.tile([128, C], BF16, tag="xkv", bufs=12, name=f"xe{i}")
                nc.sync.dma_start(t[:], xe[i * 128:(i + 1) * 128, :])
                xe_tiles.append(t)
            avT = _attention(nc, tc, sb, wpool, ln1T_own, ln1T, NT_KV,
                             d_in["wqk_sa"], d_in["wv_sa"],
                             brow("bq_sa"), brow("bk_sa"), brow("bv_sa"),
                             mask_tiles, ident_t, ones_row, ones16, "sa",
                             vflag=vflag, mask_from=NT_OWN)
            ln2T = _layer_norm_T(nc, sb, stats, xe_tiles, NT_KV, "l2", eps_ap[:])
            x1 = [sb.tile([128, C], F32, tag="res1", bufs=NT_OWN, name=f"x1_{i}")
                  for i in range(NT_OWN)]
            with tc.tile_pool(name="pspr1", bufs=4, space="PSUM") as ps_pr:
                _proj_residual(nc, sb, wpool, ps_pr, avT, d_in["wp_sa"],
                               brow("bp_sa"), x_tiles[NT_OWN:], x1,
                               ones_row, "sa")

            # ---------------- CA ----------------
            ln3T = _layer_norm_T(nc, sb, stats, x1, NT_OWN, "l3", eps_ap[:])
            avT2 = _attention(nc, tc, sb, wpool, ln3T, ln2T, NT_KV,
                              d_in["wqk_ca"], d_in["wv_ca"],
                              brow("bq_ca"), brow("bk_ca"), brow("bv_ca"),
                              None, ident_t, ones_row, ones16, "ca")
            x2 = [sb.tile([128, C], F32, tag="res2", bufs=NT_OWN, name=f"x2_{i}")
                  for i in range(NT_OWN)]
            with tc.tile_pool(name="pspr2", bufs=4, space="PSUM") as ps_pr:
                _proj_residual(nc, sb, wpool, ps_pr, avT2, d_in["wp_ca"],
                               brow("bp_ca"), x1, x2, ones_row, "ca")

            # ---------------- FFN ----------------
            ln4T = _layer_norm_T(nc, sb, stats, x2, NT_OWN, "l4", eps_ap[:])
            b1r = brow("b1")
            w1_t = _load_w512(nc, wpool, d_in["w1"], "w1")
            gT = []
            with tc.tile_pool(name="psh", bufs=4, space="PSUM") as ps_h:
                for hidt in range(NC_T):
                    ps = ps_h.tile([128, TOWN], F32, tag="h", name=f"psh{hidt}")
                    bias_mm = None
                    if b1r is not None:
                        def bias_mm(ps=ps, hidt=hidt):
                            nc.tensor.matmul(
                                ps[:], b1r[:, hidt * 128:(hidt + 1) * 128],
                                ones_row[:], start=False, stop=True,
                                skip_group_check=True)
                    _ksplit_chain(
                        nc, ps[:],
                        lambda k, hidt=hidt: w1_t[k][hidt // 4][
                            :, (hidt % 4) * 128:(hidt % 4 + 1) * 128],
                        lambda k: ln4T[k][:], NC_T, bias_mm)
                    g = sb.tile([128, TOWN], BF16, tag="gT", bufs=NC_T,
                                name=f"g{hidt}")
                    nc.scalar.activation(g[:], ps[:], AF.Gelu)
                    gT.append(g)
            out_sb = [sb.tile([128, C], F32, tag="res1", bufs=NT_OWN,
                              name=f"osb{i}") for i in range(NT_OWN)]
            with tc.tile_pool(name="psf", bufs=4, space="PSUM") as ps_f:
                _proj_residual(nc, sb, wpool, ps_f, gT, d_in["w2"],
                               brow("b2"), x2, out_sb, ones_row, "f")
            for tqt in range(NT_OWN):
                nc.sync.dma_start(out_d[tqt * 128:(tqt + 1) * 128, :],
                                  out_sb[tqt][:])
    nc.compile()
    return nc


_CACHED = {}


def _get_nc(with_bias):
    if with_bias not in _CACHED:
        _CACHED[with_bias] = build(with_bias)
    return _CACHED[with_bias]


def _stack_heads(w):
    return np.ascontiguousarray(np.transpose(np.asarray(w), (1, 0, 2))
                                .reshape(C, H * HD))


def _pack512(w):
    """[C, C] -> [2048, 512]: block ci*2+nch = w[ci*128:+128, nch*512:+512]."""
    return np.ascontiguousarray(
        w.reshape(NC_T, 128, 2, 512).transpose(0, 2, 1, 3).reshape(2048, 512))


def _packqk(wq, wk):
    """Two [C, C] -> [8192, 256]: block hp*8+ci = [wq_blk | wk_blk]."""
    q = wq.reshape(NC_T, 128, NHP, 128).transpose(2, 0, 1, 3)
    k = wk.reshape(NC_T, 128, NHP, 128).transpose(2, 0, 1, 3)
    return np.ascontiguousarray(
        np.concatenate([q, k], axis=3).reshape(NHP * NC_T * 128, 256))


def prepare_in_maps(inputs):
    inp = {k: np.asarray(v, dtype=np.float32) for k, v in inputs.items()}
    g1, be1 = inp["g1"], inp["be1"]
    g2, be2 = inp["g2"], inp["be2"]
    g3, be3 = inp["g3"], inp["be3"]
    g4, be4 = inp["g4"], inp["be4"]

    wq_sa = _stack_heads(inp["Wq_sa"]); wk_sa = _stack_heads(inp["Wk_sa"])
    wv_sa = _stack_heads(inp["Wv_sa"])
    wq_ca = _stack_heads(inp["Wq_ca"]); wk_ca = _stack_heads(inp["Wk_ca"])
    wv_ca = _stack_heads(inp["Wv_ca"])

    biases = {
        "bq_sa": be1 @ wq_sa, "bk_sa": be1 @ wk_sa, "bv_sa": be1 @ wv_sa,
        "bp_sa": inp["bp_sa"],
        "bq_ca": be3 @ wq_ca, "bk_ca": be2 @ wk_ca, "bv_ca": be2 @ wv_ca,
        "bp_ca": inp["bp_ca"],
        "b1": inp["b1"] + be4 @ inp["W1"], "b2": inp["b2"],
    }
    with_bias = any(np.abs(v).max() > 0 for v in biases.values())

    shared = {
        "ident": np.eye(128, dtype=np.float32),
        "ones16": np.ones((128, 16), np.float32),
        "wqk_sa": _packqk(g1[:, None] * wq_sa, g1[:, None] * wk_sa),
        "wv_sa": _pack512(g1[:, None] * wv_sa),
        "wp_sa": _pack512(inp["Wp_sa"]),
        "wqk_ca": _packqk(g3[:, None] * wq_ca, g2[:, None] * wk_ca),
        "wv_ca": _pack512(g2[:, None] * wv_ca),
        "wp_ca": _pack512(inp["Wp_ca"]),
        "w1": _pack512(g4[:, None] * inp["W1"]),
        "w2": _pack512(inp["W2"]),
    }
    if with_bias:
        shared["ones_row"] = np.ones((1, 512), np.float32)
        for k, v in biases.items():
            shared[k] = v.reshape(1, C)
    shared = {k: np.ascontiguousarray(v.astype(BF)) for k, v in shared.items()}

    x = inp["x"]; xe = inp["x_encode"]
    in_maps = []
    for core in range(NCORES):
        b = core // 2
        half = core % 2
        own = slice(half * TOWN, (half + 1) * TOWN)
        other = slice((1 - half) * TOWN, (2 - half) * TOWN)
        # kv layout: [other half | own half]; own queries are rows 512..1023
        x_perm = np.concatenate([x[b, other, :], x[b, own, :]], axis=0)
        # other half: fully visible for half=1 (keys before queries), fully
        # hidden for half=0 -> v-flag 0/1; own half: shared triangle mask
        vf = np.zeros(T, np.float32)
        vf[:TOWN] = 1.0 if half == 1 else 0.0
        vf[TOWN:] = 1.0
        vf = np.ascontiguousarray(vf.reshape(NT_KV, 128).T)
        tl = np.arange(128)
        m = np.where(tl[:, None] <= tl[None, :], 0.0, -30000.0)
        im = dict(shared)
        im["x_kv"] = np.ascontiguousarray(x_perm, dtype=np.float32)
        im["xe"] = np.ascontiguousarray(xe[b].astype(BF))
        im["maskT"] = np.ascontiguousarray(m.astype(BF))
        im["vflag"] = vf
        in_maps.append(im)
    return in_maps, with_bias


def run(inputs, trace=False, **kw):
    in_maps, with_bias = prepare_in_maps(inputs)
    nc = _get_nc(with_bias)
    r = bass_utils.run_bass_kernel_spmd(nc, in_maps, core_ids=list(range(NCORES)),
                                        trace=trace, **kw)
    out = np.empty((B, T, C), np.float32)
    for core in range(NCORES):
        b = core // 2
        half = core % 2
        out[b, half * TOWN:(half + 1) * TOWN, :] = r.results[core]["out"]
    return out, r


def kernel(**inputs):
    out, _ = run(inputs)
    return out


if __name__ == "__main__":
    build()
    print("build ok")



# revision 19
# speedup vs baseline: 1.7225x; 1.1616x over previous
#!/usr/bin/env python3
"""Trainium2 Bass kernel for nn_DecoderBlock (B=4,T=1024,C=1024,H=16,FFN=C).

Sharding: 8 NeuronCores, zero collectives. Core c owns 512 query tokens of
batch b=c//2 (half h=c%2) and computes the whole decoder block for them
end-to-end: causal self-attention over its batch's 1024 keys (full rectangle
+ additive mask; the host permutes tokens so the owned 512 always sit first,
giving a single SPMD program), cross-attention over the encoder, FFN. The
host splits inputs and reassembles the output.

On-device: natural-layout residual stream [tokens, C] in fp32. LayerNorm
stats+apply run in natural layout (per-partition scalars); the [C, tokens]
operand every projection needs is produced by DMA-engine transposes (bf16),
keeping the PE free. Matmul path is bf16 with fp32 PSUM accumulation and an
fp32 residual stream. Wide-K projection chains are split into two K=64
row-group halves running concurrently on the PE (hides LDWEIGHTS + pipe
drain). Attention runs in S^T layout [tk, tq]: the two heads of a pair are
row-group packed K=64 matmuls; exp runs on the scalar engine straight out
of PSUM (scale=1/8 fused); the causal mask is added into PSUM via an
identity-matmul; softmax sums come from an appended ones-column in V (M=65
matmuls); the division is an approx-reciprocal + gpsimd partition-broadcast
fused multiply. LN gamma/beta are folded into the weights on the host;
projection biases are all zero in this problem (verified at runtime) and
skipped, with a rank-1 matmul fallback otherwise.
"""
import sys
if "/opt/trn_rl_repo" not in sys.path:
    sys.path.insert(0, "/opt/trn_rl_repo")

import numpy as np
import ml_dtypes

import concourse.bass as bass
import concourse.mybir as mybir
import concourse.tile as tile
from concourse import bacc
from concourse import bass_utils

B, T, TE, C, H, HD = 4, 1024, 1024, 1024, 16, 64
NCORES = 8
TOWN = T // 2
EPS = 1e-5
F32 = mybir.dt.float32
BF16 = mybir.dt.bfloat16
AF = mybir.ActivationFunctionType
ALU = mybir.AluOpType
AXX = mybir.AxisListType.X
NT_KV = T // 128
NT_OWN = TOWN // 128
NC_T = C // 128
NHP = H // 2
BF = np.dtype(ml_dtypes.bfloat16)
import os
USE_DMA_T = os.environ.get("USE_DMA_T", "0") == "1"
USE_KSPLIT = os.environ.get("USE_KSPLIT", "0") == "1"


def _ksplit_chain(nc, ps, lhs_tiles, rhs_tiles, n_k, bias_mm=None):
    """Accumulate sum_k lhs[k].T @ rhs[k] into psum `ps` as two concurrent
    K=64 row-group chains. lhs_tiles/rhs_tiles: callables k -> AP."""
    if not USE_KSPLIT:
        for k in range(n_k):
            last = (k == n_k - 1) and bias_mm is None
            nc.tensor.matmul(ps, lhs_tiles(k)[:], rhs_tiles(k)[:],
                             start=(k == 0), stop=last, skip_group_check=True)
        if bias_mm is not None:
            bias_mm()
        return
    for k in range(n_k):
        l = lhs_tiles(k); r = rhs_tiles(k)
        nc.tensor.matmul(ps, l[0:64, :], r[0:64, :], start=(k == 0), stop=False,
                         skip_group_check=True)
    for k in range(n_k):
        l = lhs_tiles(k); r = rhs_tiles(k)
        last = (k == n_k - 1) and bias_mm is None
        nc.tensor.matmul(ps, l[64:128, :], r[64:128, :], start=False, stop=last,
                         skip_group_check=True)
    if bias_mm is not None:
        bias_mm()


def _layer_norm_T(nc, sb, stats, x_tiles, n_t, pfx, eps_ap):
    """LN over n_t natural [128, C] tiles -> NC_T transposed bf16 tiles
    [128, n_t*128]. Stats chains are per-tile so tile i's transposes are
    unblocked as soon as tile i's data (not the whole LN) is ready."""
    out = [sb.tile([128, n_t * 128], BF16, tag=f"lnT{n_t}", bufs=(10 if n_t == NT_KV else NC_T),
                   name=f"lnT{pfx}{ci}") for ci in range(NC_T)]
    # mean+var via DVE bn_stats/bn_aggr (keeps Square/Ln off the scalar
    # engine -> no activation-table thrash); one vectorized Rsqrt per group.
    mv = stats.tile([128, n_t, 2], F32, tag="mv", bufs=2, name=f"mv{pfx}")
    rstd = stats.tile([128, n_t], F32, tag="rstd", bufs=2, name=f"rstd{pfx}")
    for i in range(n_t):
        bns = stats.tile([128, 12], F32, tag="bns", bufs=8, name=f"bns{pfx}{i}")
        for c in range(2):
            nc.vector.bn_stats(bns[:, c * 6:(c + 1) * 6],
                               x_tiles[i][:, c * 512:(c + 1) * 512])
        nc.vector.bn_aggr(mv[:, i, :], bns[:])
    half = (n_t + 1) // 2
    with tc_ref[0].tile_pool(name=f"lnp{pfx}", bufs=4, space="PSUM") as lnp:
        for g0 in range(0, n_t, half):
            g1 = min(g0 + half, n_t)
            sstd = stats.tile([128, n_t], F32, tag="sstd", bufs=2,
                              name=f"sstd{pfx}{g0}")
            nc.scalar.activation(sstd[:, g0:g1], mv[:, g0:g1, 1], AF.Sqrt,
                                 bias=eps_ap)
            nc.vector.reciprocal(rstd[:, g0:g1], sstd[:, g0:g1])
            ln_nat = []
            for i in range(g0, g1):
                t = sb.tile([128, C], BF16, tag="lnnat", bufs=6, name=f"nat{pfx}{i}")
                nc.vector.tensor_scalar(t[:], x_tiles[i][:], mv[:, i, 0:1],
                                        rstd[:, i:i + 1], op0=ALU.subtract,
                                        op1=ALU.mult)
                ln_nat.append(t)
            for ci in range(NC_T):
                ps = lnp.tile([128, half * 128], BF16, tag="lnp",
                              name=f"lnps{pfx}{ci}{g0}")
                for j in range(g1 - g0):
                    # Transpose as a PLAIN matmul (data stationary, identity
                    # streaming): out = data.T @ I. ~3x faster than
                    # transpose-mode (FWL weight load, pipelined MMs) and it
                    # counts as PE activity for the HAM clock gate.
                    nc.tensor.matmul(ps[:, j * 128:(j + 1) * 128],
                                     ln_nat[j][:, ci * 128:(ci + 1) * 128],
                                     ident_ref[0][:], start=True, stop=True,
                                     skip_group_check=True)
                nc.vector.tensor_copy(out[ci][:, g0 * 128:g1 * 128],
                                      ps[:, 0:(g1 - g0) * 128])
    return out


tc_ref = [None]
ident_ref = [None]


def _load_w512(nc, wpool, dram_ap, pfx):
    """dram_ap is host-packed [NC_T*2*128, 512]: block ci*2+nch holds the
    [128, 512] tile (contiguous 128 KiB -> large DMA descriptors)."""
    tiles = []
    for ci in range(NC_T):
        row = []
        for nch in range(2):
            b = ci * 2 + nch
            t = wpool.tile([128, 512], BF16, tag="w512", bufs=16,
                           name=f"w{pfx}{ci}_{nch}")
            nc.sync.dma_start(t[:], dram_ap[b * 128:(b + 1) * 128, :])
            row.append(t)
        tiles.append(row)
    return tiles


def _attention(nc, tc, sb, wpool, src_qT, src_kvT, nt_k, w_qk, w_v,
               b_q, b_k, b_v, mask_tiles, ident_t, ones_row, ones16, tag,
               vflag=None, mask_from=0):
    """One MHA. Returns avT: NHP x [128, TOWN] bf16 (softmax-normalized).
    vflag: optional [T,1] f32 sbuf tile; V rows (and their ones column) are
    multiplied by it, implementing all-or-nothing key masking per 128-block.
    mask_tiles[i] is applied (identity-matmul add) only for i >= mask_from."""
    nk_cols = nt_k * 128

    # V projection, natural [tk, d], ones column appended per head (65-stride)
    wv_t = _load_w512(nc, wpool, w_v, f"v{tag}")
    v_sb = []
    for i in range(nt_k):
        vt = sb.tile([128, H * 65], BF16, tag="vsb", bufs=NT_KV,
                     name=f"v{tag}{i}")
        ones_dst = vt[:].rearrange("p (h c) -> p h c", c=65)[:, :, 64:65]
        ones_src = ones16[:].rearrange("p (h c) -> p h c", c=1)
        if vflag is not None:
            nc.vector.tensor_scalar_mul(ones_dst, ones_src,
                                        vflag[:, i:i + 1])
        else:
            nc.vector.tensor_copy(ones_dst, ones_src)
        v_sb.append(vt)
    with tc.tile_pool(name=f"psv{tag}", bufs=4, space="PSUM") as ps_v:
        for nch in range(2):
            for i in range(nt_k):
                ps = ps_v.tile([128, 512], F32, tag="vp", name=f"psv{tag}{nch}{i}")
                bias_mm = None
                if b_v is not None:
                    def bias_mm(ps=ps, nch=nch):
                        nc.tensor.matmul(ps[:], ones_row[:, 0:128],
                                         b_v[:, nch * 512:(nch + 1) * 512],
                                         start=False, stop=True,
                                         skip_group_check=True)
                _ksplit_chain(nc, ps[:],
                              lambda k, i=i: src_kvT[k][:, i * 128:(i + 1) * 128],
                              lambda k, nch=nch: wv_t[k][nch][:],
                              NC_T, bias_mm)
                dst = v_sb[i][:, nch * 520:(nch + 1) * 520].rearrange(
                    "p (h c) -> p h c", c=65)[:, :, 0:64]
                src_ps = ps[:].rearrange("p (h c) -> p h c", c=64)
                if vflag is not None:
                    nc.vector.tensor_scalar_mul(dst, src_ps,
                                                vflag[:, i:i + 1])
                else:
                    nc.vector.tensor_copy(dst, src_ps)

    with (
        tc.tile_pool(name=f"psqk{tag}", bufs=2, space="PSUM") as ps_qk,
        tc.tile_pool(name=f"pssc{tag}", bufs=4, space="PSUM") as ps_sc,
        tc.tile_pool(name=f"psav{tag}", bufs=2, space="PSUM") as ps_av,
    ):
        avT = _attn_heads(nc, sb, wpool, ps_qk, ps_sc, ps_av, src_qT, src_kvT,
                          nt_k, w_qk, b_q, b_k, mask_tiles, ident_t,
                          ones_row, v_sb, tag, mask_from)
    return avT


def _attn_heads(nc, sb, wpool, ps_qk, ps_sc, ps_av, src_qT, src_kvT, nt_k,
                w_qk, b_q, b_k, mask_tiles, ident_t, ones_row, v_sb, tag,
                mask_from=0):
    nk_cols = nt_k * 128
    avT = []
    for hp in range(NHP):
        # host-packed [8192, 256]: block hp*8+ci = [wq | wk] for (hp, ci)
        wq_t, wk_t = [], []
        for ci in range(NC_T):
            b = hp * NC_T + ci
            tqk = wpool.tile([128, 256], BF16, tag="w256", bufs=16,
                             name=f"wqk{tag}{hp}_{ci}")
            nc.sync.dma_start(tqk[:], w_qk[b * 128:(b + 1) * 128, :])
            wq_t.append(tqk[:, 0:128])
            wk_t.append(tqk[:, 128:256])

        qT = sb.tile([128, TOWN], BF16, tag="qT", bufs=3, name=f"qT{tag}{hp}")
        ps = ps_qk.tile([128, TOWN], F32, tag="qk", name=f"psq{tag}{hp}")
        bias_mm = None
        if b_q is not None:
            def bias_mm(ps=ps, hp=hp):
                nc.tensor.matmul(ps[:], b_q[:, hp * 128:(hp + 1) * 128],
                                 ones_row[:], start=False, stop=True,
                                 skip_group_check=True)
        _ksplit_chain(nc, ps[:], lambda k: wq_t[k][:], lambda k: src_qT[k][:],
                      NC_T, bias_mm)
        nc.vector.tensor_copy(qT[:], ps[:])

        kT = sb.tile([128, nk_cols], BF16, tag="kT", bufs=2, name=f"kT{tag}{hp}")
        for half in range(nk_cols // 512):
            ps = ps_qk.tile([128, 512], F32, tag="qk", name=f"psk{tag}{hp}{half}")
            bias_mm = None
            if b_k is not None:
                def bias_mm(ps=ps, hp=hp):
                    nc.tensor.matmul(ps[:], b_k[:, hp * 128:(hp + 1) * 128],
                                     ones_row[:], start=False, stop=True,
                                     skip_group_check=True)
            _ksplit_chain(nc, ps[:], lambda k: wk_t[k][:],
                          lambda k, half=half: src_kvT[k][:, half * 512:(half + 1) * 512],
                          NC_T, bias_mm)
            nc.vector.tensor_copy(kT[:, half * 512:(half + 1) * 512], ps[:])

        at = sb.tile([128, TOWN], BF16, tag="avT", bufs=NHP, name=f"avT{tag}{hp}")
        pav = [ps_av.tile([65, TOWN], F32, tag="av", name=f"psav{tag}{hp}{s}")
               for s in range(2)]
        for i in range(nt_k):
            psc = [ps_sc.tile([128, TOWN], F32, tag="sc", name=f"pssc{tag}{hp}{i}{s}")
                   for s in range(2)]
            has_mask = mask_tiles is not None and i >= mask_from
            # Causal own-key tile i': query cols < i'*128 are fully hidden
            # (skip), col block i' is the triangular diagonal (add tri mask),
            # cols > (i'+1)*128 fully visible.
            j0 = (i - mask_from) * 128 if has_mask else 0
            for sub in range(2):
                nc.tensor.matmul(psc[sub][:, j0:TOWN],
                                 kT[sub * 64:(sub + 1) * 64, i * 128:(i + 1) * 128],
                                 qT[sub * 64:(sub + 1) * 64, j0:TOWN],
                                 start=True, stop=not has_mask,
                                 skip_group_check=True)
            if has_mask:
                for sub in range(2):
                    nc.tensor.matmul(psc[sub][:, j0:j0 + 128], ident_t[:],
                                     mask_tiles[0][:],
                                     start=False, stop=True,
                                     skip_group_check=True)
            for sub in range(2):
                h = hp * 2 + sub
                es = sb.tile([128, TOWN], BF16, tag="exp", bufs=5,
                             name=f"es{tag}{h}{i}")
                nc.scalar.activation(es[:, j0:TOWN], psc[sub][:, j0:TOWN],
                                     AF.Exp, scale=float(HD) ** -0.5)
                nc.tensor.matmul(pav[sub][:, j0:TOWN],
                                 v_sb[i][:, h * 65:h * 65 + 65],
                                 es[:, j0:TOWN], start=(i == 0),
                                 stop=(i == nt_k - 1),
                                 skip_group_check=True)
        for sub in range(2):
            h = hp * 2 + sub
            s_h = sb.tile([1, TOWN], F32, tag="sums", bufs=2, name=f"s{tag}{h}")
            nc.vector.tensor_copy(s_h[:], pav[sub][64:65, :])
            rec = sb.tile([1, TOWN], F32, tag="rec", bufs=2, name=f"rec{tag}{h}")
            nc.vector.reciprocal_approx_fast(rec[:], s_h[:])
            r_bc = sb.tile([64, TOWN], F32, tag="rbc", bufs=2, name=f"rb{tag}{h}")
            nc.gpsimd.partition_broadcast(r_bc[:], rec[:])
            nc.vector.tensor_mul(at[sub * 64:(sub + 1) * 64, :],
                                 pav[sub][0:64, :], r_bc[:])
        avT.append(at)
    return avT


def _proj_residual(nc, sb, wpool, ps_pool, lhsT_tiles, w_dram, b_row,
                   resid_tiles, out_tiles, ones_row, tag):
    """out[tq, c'] = lhsT.T @ W + bias + resid (fp32 out)."""
    w_t = _load_w512(nc, wpool, w_dram, f"p{tag}")
    for nch in range(2):
        for tqt in range(NT_OWN):
            ps = ps_pool.tile([128, 512], F32, tag="pr", name=f"pr{tag}{nch}{tqt}")
            bias_mm = None
            if b_row is not None:
                def bias_mm(ps=ps, nch=nch):
                    nc.tensor.matmul(ps[:], ones_row[:, 0:128],
                                     b_row[:, nch * 512:(nch + 1) * 512],
                                     start=False, stop=True,
                                     skip_group_check=True)
            _ksplit_chain(nc, ps[:],
                          lambda k, tqt=tqt: lhsT_tiles[k][:, tqt * 128:(tqt + 1) * 128],
                          lambda k, nch=nch: w_t[k][nch][:],
                          NC_T, bias_mm)
            nc.vector.tensor_add(out_tiles[tqt][:, nch * 512:(nch + 1) * 512],
                                 ps[:], resid_tiles[tqt][:, nch * 512:(nch + 1) * 512])


def build(with_bias=False):
    nc = bacc.Bacc("TRN2", target_bir_lowering=False, debug=False,
                   num_devices=NCORES)
    d_in = {}

    def din(name, shape, dt=BF16):
        d_in[name] = nc.dram_tensor(name, shape, dt, kind="ExternalInput").ap()
        return d_in[name]

    x_kv = din("x_kv", [T, C], F32)
    xe = din("xe", [TE, C])
    maskT = din("maskT", [128, 128])
    vflag_d = din("vflag", [128, NT_KV], F32)
    ident = din("ident", [128, 128])
    ones16_d = din("ones16", [128, 16])
    for w in ["wv_sa", "wp_sa", "wv_ca", "wp_ca", "w1", "w2"]:
        din(w, [NC_T * 2 * 128, 512])
    for w in ["wqk_sa", "wqk_ca"]:
        din(w, [NHP * NC_T * 128, 256])
    bias_names = ["bq_sa", "bk_sa", "bv_sa", "bp_sa", "bq_ca", "bk_ca",
                  "bv_ca", "bp_ca", "b1", "b2"]
    if with_bias:
        ones_row_d = din("ones_row", [1, 512])
        for b in bias_names:
            din(b, [1, C])
    out_d = nc.dram_tensor("out", [TOWN, C], F32, kind="ExternalOutput").ap()

    with tile.TileContext(nc) as tc:
        with (
            tc.tile_pool(name="sb", bufs=1) as sb,
            tc.tile_pool(name="stats", bufs=1) as stats,
            tc.tile_pool(name="wpool", bufs=1) as wpool,
        ):
            ident_t = sb.tile([128, 128], BF16, tag="ident", name="identt")
            nc.sync.dma_start(ident_t[:], ident)
            tc_ref[0] = tc
            ident_ref[0] = ident_t
            ones16 = sb.tile([128, 16], BF16, tag="ones16", name="ones16t")
            nc.sync.dma_start(ones16[:], ones16_d)
            eps_ap = sb.tile([128, 1], F32, tag="epsap", name="epst")
            nc.gpsimd.memset(eps_ap[:], EPS)
            if with_bias:
                ones_row = sb.tile([1, 512], BF16, tag="onesrow", name="onesrowt")
                nc.sync.dma_start(ones_row[:], ones_row_d)

                def brow(name):
                    t = sb.tile([1, C], BF16, tag="brow", bufs=4, name=f"br{name}")
                    nc.sync.dma_start(t[:], d_in[name])
                    return t
            else:
                ones_row = None
                brow = lambda name: None

            x_tiles = []
            for i in range(NT_KV):
                t = sb.tile([128, C], F32, tag="xkv", bufs=12, name=f"x{i}")
                nc.sync.dma_start(t[:], x_kv[i * 128:(i + 1) * 128, :])
                x_tiles.append(t)
            tri = sb.tile([128, 128], BF16, tag="mask", name="tri")
            nc.sync.dma_start(tri[:], maskT)
            mask_tiles = [tri]
            vflag = sb.tile([128, NT_KV], F32, tag="vflag", name="vflagt")
            nc.sync.dma_start(vflag[:], vflag_d)

            # ---------------- SA (+ ln2 emitted early for overlap) --------
            ln1T = _layer_norm_T(nc, sb, stats, x_tiles, NT_KV, "l1", eps_ap[:])
            ln1T_own = [t[:, TOWN:] for t in ln1T]  # own tokens = rows 512..1023
            xe_tiles = []
            for i in range(NT_KV):
                t = sb.tile([128, C], BF16, tag="xkv", bufs=12, name=f"xe{i}")
                nc.sync.dma_start(t[:], xe[i * 128:(i + 1) * 128, :])
                xe_tiles.append(t)
            avT = _attention(nc, tc, sb, wpool, ln1T_own, ln1T, NT_KV,
                             d_in["wqk_sa"], d_in["wv_sa"],
                             brow("bq_sa"), brow("bk_sa"), brow("bv_sa"),
                             mask_tiles, ident_t, ones_row, ones16, "sa",
                             vflag=vflag, mask_from=NT_OWN)
            ln2T = _layer_norm_T(nc, sb, stats, xe_tiles, NT_KV, "l2", eps_ap[:])
            x1 = [sb.tile([128, C], F32, tag="res1", bufs=NT_OWN, name=f"x1_{i}")
                  for i in range(NT_OWN)]
            with tc.tile_pool(name="pspr1", bufs=4, space="PSUM") as ps_pr:
                _proj_residual(nc, sb, wpool, ps_pr, avT, d_in["wp_sa"],
                               brow("bp_sa"), x_tiles[NT_OWN:], x1,
                               ones_row, "sa")

            # ---------------- CA ----------------
            ln3T = _layer_norm_T(nc, sb, stats, x1, NT_OWN, "l3", eps_ap[:])
            avT2 = _attention(nc, tc, sb, wpool, ln3T, ln2T, NT_KV,
                              d_in["wqk_ca"], d_in["wv_ca"],
                              brow("bq_ca"), brow("bk_ca"), brow("bv_ca"),
                              None, ident_t, ones_row, ones16, "ca")
            x2 = [sb.tile([128, C], F32, tag="res2", bufs=NT_OWN, name=f"x2_{i}")
                  for i in range(NT_OWN)]
            with tc.tile_pool(name="pspr2", bufs=4, space="PSUM") as ps_pr:
                _proj_residual(nc, sb, wpool, ps_pr, avT2, d_in["wp_ca"],
                               brow("bp_ca"), x1, x2, ones_row, "ca")

            # ---------------- FFN ----------------
            ln4T = _layer_norm_T(nc, sb, stats, x2, NT_OWN, "l4", eps_ap[:])
            b1r = brow("b1")
            w1_t = _load_w512(nc, wpool, d_in["w1"], "w1")
            gT = []
            with tc.tile_pool(name="psh", bufs=4, space="PSUM") as ps_h:
                for hidt in range(NC_T):
                    ps = ps_h.tile([128, TOWN], F32, tag="h", name=f"psh{hidt}")
                    bias_mm = None
                    if b1r is not None:
                        def bias_mm(ps=ps, hidt=hidt):
                            nc.tensor.matmul(
                                ps[:], b1r[:, hidt * 128:(hidt + 1) * 128],
                                ones_row[:], start=False, stop=True,
                                skip_group_check=True)
                    _ksplit_chain(
                        nc, ps[:],
                        lambda k, hidt=hidt: w1_t[k][hidt // 4][
                            :, (hidt % 4) * 128:(hidt % 4 + 1) * 128],
                        lambda k: ln4T[k][:], NC_T, bias_mm)
                    g = sb.tile([128, TOWN], BF16, tag="gT", bufs=NC_T,
                                name=f"g{hidt}")
                    nc.scalar.activation(g[:], ps[:], AF.Gelu)
                    gT.append(g)
            out_sb = [sb.tile([128, C], F32, tag="res1", bufs=NT_OWN,
                              name=f"osb{i}") for i in range(NT_OWN)]
            with tc.tile_pool(name="psf", bufs=4, space="PSUM") as ps_f:
                _proj_residual(nc, sb, wpool, ps_f, gT, d_in["w2"],
                               brow("b2"), x2, out_sb, ones_row, "f")
            for tqt in range(NT_OWN):
                nc.sync.dma_start(out_d[tqt * 128:(tqt + 1) * 128, :],
                                  out_sb[tqt][:])
    nc.compile()
    return nc


_CACHED = {}


def _get_nc(with_bias):
    if with_bias not in _CACHED:
        _CACHED[with_bias] = build(with_bias)
    return _CACHED[with_bias]


def _stack_heads(w):
    return np.ascontiguousarray(np.transpose(np.asarray(w), (1, 0, 2))
                                .reshape(C, H * HD))


def _pack512(w):
    """[C, C] -> [2048, 512]: block ci*2+nch = w[ci*128:+128, nch*512:+512]."""
    return np.ascontiguousarray(
        w.reshape(NC_T, 128, 2, 512).transpose(0, 2, 1, 3).reshape(2048, 512))


def _packqk(wq, wk):
    """Two [C, C] -> [8192, 256]: block hp*8+ci = [wq_blk | wk_blk]."""
    q = wq.reshape(NC_T, 128, NHP, 128).transpose(2, 0, 1, 3)
    k = wk.reshape(NC_T, 128, NHP, 128).transpose(2, 0, 1, 3)
    return np.ascontiguousarray(
        np.concatenate([q, k], axis=3).reshape(NHP * NC_T * 128, 256))


def prepare_in_maps(inputs):
    inp = {k: np.asarray(v, dtype=np.float32) for k, v in inputs.items()}
    g1, be1 = inp["g1"], inp["be1"]
    g2, be2 = inp["g2"], inp["be2"]
    g3, be3 = inp["g3"], inp["be3"]
    g4, be4 = inp["g4"], inp["be4"]

    wq_sa = _stack_heads(inp["Wq_sa"]); wk_sa = _stack_heads(inp["Wk_sa"])
    wv_sa = _stack_heads(inp["Wv_sa"])
    wq_ca = _stack_heads(inp["Wq_ca"]); wk_ca = _stack_heads(inp["Wk_ca"])
    wv_ca = _stack_heads(inp["Wv_ca"])

    biases = {
        "bq_sa": be1 @ wq_sa, "bk_sa": be1 @ wk_sa, "bv_sa": be1 @ wv_sa,
        "bp_sa": inp["bp_sa"],
        "bq_ca": be3 @ wq_ca, "bk_ca": be2 @ wk_ca, "bv_ca": be2 @ wv_ca,
        "bp_ca": inp["bp_ca"],
        "b1": inp["b1"] + be4 @ inp["W1"], "b2": inp["b2"],
    }
    with_bias = any(np.abs(v).max() > 0 for v in biases.values())

    shared = {
        "ident": np.eye(128, dtype=np.float32),
        "ones16": np.ones((128, 16), np.float32),
        "wqk_sa": _packqk(g1[:, None] * wq_sa, g1[:, None] * wk_sa),
        "wv_sa": _pack512(g1[:, None] * wv_sa),
        "wp_sa": _pack512(inp["Wp_sa"]),
        "wqk_ca": _packqk(g3[:, None] * wq_ca, g2[:, None] * wk_ca),
        "wv_ca": _pack512(g2[:, None] * wv_ca),
        "wp_ca": _pack512(inp["Wp_ca"]),
        "w1": _pack512(g4[:, None] * inp["W1"]),
        "w2": _pack512(inp["W2"]),
    }
    if with_bias:
        shared["ones_row"] = np.ones((1, 512), np.float32)
        for k, v in biases.items():
            shared[k] = v.reshape(1, C)
    shared = {k: np.ascontiguousarray(v.astype(BF)) for k, v in shared.items()}

    x = inp["x"]; xe = inp["x_encode"]
    in_maps = []
    for core in range(NCORES):
        b = core // 2
        half = core % 2
        own = slice(half * TOWN, (half + 1) * TOWN)
        other = slice((1 - half) * TOWN, (2 - half) * TOWN)
        # kv layout: [other half | own half]; own queries are rows 512..1023
        x_perm = np.concatenate([x[b, other, :], x[b, own, :]], axis=0)
        # other half: fully visible for half=1 (keys before queries), fully
        # hidden for half=0 -> v-flag 0/1; own half: shared triangle mask
        vf = np.zeros(T, np.float32)
        vf[:TOWN] = 1.0 if half == 1 else 0.0
        vf[TOWN:] = 1.0
        vf = np.ascontiguousarray(vf.reshape(NT_KV, 128).T)
        tl = np.arange(128)
        m = np.where(tl[:, None] <= tl[None, :], 0.0, -30000.0)
        im = dict(shared)
        im["x_kv"] = np.ascontiguousarray(x_perm, dtype=np.float32)
        im["xe"] = np.ascontiguousarray(xe[b].astype(BF))
        im["maskT"] = np.ascontiguousarray(m.astype(BF))
        im["vflag"] = vf
        in_maps.append(im)
    return in_maps, with_bias


def run(inputs, trace=False, **kw):
    in_maps, with_bias = prepare_in_maps(inputs)
    nc = _get_nc(with_bias)
    r = bass_utils.run_bass_kernel_spmd(nc, in_maps, core_ids=list(range(NCORES)),
                                        trace=trace, **kw)
    out = np.empty((B, T, C), np.float32)
    for core in range(NCORES):
        b = core // 2
        half = core % 2
        out[b, half * TOWN:(half + 1) * TOWN, :] = r.results[core]["out"]
    return out, r


def kernel(**inputs):
    out, _ = run(inputs)
    return out


if __name__ == "__main__":
    build()
    print("build ok")

